# revision 24
# baseline (speedup 1.0000x reference)
"""TRN2 Bass kernel for batched dot-product attention (no scale, eval mode).

reference:
    score   = einsum('bqd,bvd->bqv', query, value)      # B=16, L=2048, D=1024
    attn    = softmax(score, axis=-1)
    context = einsum('bqv,bvd->bqd', attn, value)

Sharding: data-parallel over batch; each of 8 NeuronCores handles 2 batch
elements, no communication. Inputs are pre-cast to fp16 on the host (part of
the sharding/layout prep); matmuls run fp16 with fp32 PSUM accumulation.

Per-core per-batch plan:
  - preload V: natural fp16 copy Vn + transposed copy VT via PE transposes
  - per 128-row q-tile (software-pipelined, tail lags one tile):
      head: q tile load, QT via PE transposes, S = QT.T @ VT -> PSUM,
            per-chunk rowmax on DVE
      tail: exp(S - max) on ACT (fp16 P, fused rowsum), P^T via PE
            transposes, O = PT.T @ Vn -> PSUM, multiply by 1/rowsum, DMA out
"""

from contextlib import ExitStack

import numpy as np

import concourse.tile as tile
from concourse import bacc, mybir
from concourse.bass_utils import run_bass_kernel_spmd
from concourse.masks import make_identity

B, LQ, LV, D = 16, 2048, 2048, 1024
NCORES = 8
BPC = B // NCORES  # batches per core
P = 128
NQT = LQ // P  # 16 q tiles
NVT = LV // P  # 16 v tiles
ND = D // P  # 8 d tiles
VCH = 512  # MM1 moving-operand chunk (free dim)
NCH = LV // VCH  # 4
DCH = 512  # MM2 moving-operand chunk
NDCH = D // DCH  # 2

f32 = mybir.dt.float32
f16 = mybir.dt.float16
EXP = mybir.ActivationFunctionType.Exp
AX = mybir.AxisListType.X


def build_nc():
    nc = bacc.Bacc("TRN2", target_bir_lowering=False, debug=False)
    q_d = nc.dram_tensor("q16", [BPC, LQ, D], f16, kind="ExternalInput").ap()
    v_d = nc.dram_tensor("v16", [BPC, LV, D], f16, kind="ExternalInput").ap()
    o_d = nc.dram_tensor("o", [BPC, LQ, D], f32, kind="ExternalOutput").ap()

    with tile.TileContext(nc) as tc, ExitStack() as ctx:
        const = ctx.enter_context(tc.tile_pool(name="const", bufs=1))
        vpool = ctx.enter_context(tc.tile_pool(name="vpool", bufs=2))
        stage = ctx.enter_context(tc.tile_pool(name="stage", bufs=3))
        qtp = ctx.enter_context(tc.tile_pool(name="qtp", bufs=2))
        pp = ctx.enter_context(tc.tile_pool(name="pp", bufs=2))
        ptp = ctx.enter_context(tc.tile_pool(name="ptp", bufs=2))
        outp = ctx.enter_context(tc.tile_pool(name="outp", bufs=2))
        statp = ctx.enter_context(tc.tile_pool(name="statp", bufs=2))
        psum = ctx.enter_context(tc.tile_pool(name="psum", bufs=3, space="PSUM"))
        psum_t = ctx.enter_context(tc.tile_pool(name="psum_t", bufs=2, space="PSUM"))

        ident = const.tile([P, P], f16)
        make_identity(nc, ident)

        def preload(b):
            """V natural fp16 + VT via PE transposes."""
            VT = vpool.tile([P, ND, LV], f16, tag="VT", name=f"VT{b}")
            Vn = vpool.tile([P, NVT, D], f16, tag="Vn", name=f"Vn{b}")
            for j in range(NVT):
                nc.gpsimd.dma_start(
                    out=Vn[:, j, :], in_=v_d[b, j * P : (j + 1) * P, :]
                )
                for k in range(ND):
                    pst = psum_t.tile([P, P], f16, tag="pst", name=f"pvt{b}_{j}_{k}")
                    nc.tensor.transpose(pst, Vn[:, j, k * P : (k + 1) * P], ident)
                    nc.scalar.copy(VT[:, k, j * P : (j + 1) * P], pst)
            return VT, Vn

        def head(b, qi, VT):
            """Load+transpose Q tile, S = Q @ V^T, per-chunk row maxes."""
            q16 = stage.tile([P, D], f16, tag="q16", name=f"q16_{b}_{qi}")
            nc.gpsimd.dma_start(out=q16, in_=q_d[b, qi * P : (qi + 1) * P, :])
            QT = qtp.tile([P, ND, P], f16, tag="QT", name=f"QT{b}_{qi}")
            for k in range(ND):
                pst = psum_t.tile([P, P], f16, tag="pst", name=f"pqt{b}_{qi}_{k}")
                nc.tensor.transpose(pst, q16[:, k * P : (k + 1) * P], ident)
                nc.scalar.copy(QT[:, k, :], pst)

            psS = [
                psum.tile([P, 2 * VCH], f32, tag="ps", name=f"psS{b}_{qi}_{h}")
                for h in range(NCH // 2)
            ]
            stats = statp.tile([P, NCH], f32, tag="stats", name=f"st{b}_{qi}")
            for n in range(NCH):
                sl = slice((n % 2) * VCH, (n % 2 + 1) * VCH)
                for k in range(ND):
                    nc.tensor.matmul(
                        psS[n // 2][:, sl],
                        QT[:, k, :],
                        VT[:, k, n * VCH : (n + 1) * VCH],
                        start=(k == 0),
                        stop=(k == ND - 1),
                    )
                nc.vector.reduce_max(stats[:, n : n + 1], psS[n // 2][:, sl], axis=AX)
            return psS, stats

        def softmax_part(b, qi, psS, stats):
            """Softmax on ACT/DVE: exp(S - max) -> fp16 P, plus 1/rowsum."""
            negmax = statp.tile([P, 1], f32, tag="negmax", name=f"nm{b}_{qi}")
            nc.vector.reduce_max(negmax, stats, axis=AX, negate=True)
            sums = statp.tile([P, NCH], f32, tag="sums", name=f"sm{b}_{qi}")
            Pt = pp.tile([P, LV], f16, tag="P", name=f"P{b}_{qi}")
            for n in range(NCH):
                sl = slice((n % 2) * VCH, (n % 2 + 1) * VCH)
                nc.scalar.activation(
                    Pt[:, n * VCH : (n + 1) * VCH],
                    psS[n // 2][:, sl],
                    EXP,
                    bias=negmax,
                    accum_out=sums[:, n : n + 1],
                )
            rowsum = statp.tile([P, 1], f32, tag="rowsum", name=f"rs{b}_{qi}")
            nc.vector.reduce_sum(rowsum, sums, axis=AX)
            rinv = statp.tile([P, 1], f32, tag="rinv", name=f"ri{b}_{qi}")
            nc.vector.reciprocal(rinv, rowsum)
            return Pt, rinv

        def tail(b, qi, Pt, rinv, Vn):
            """P^T via PE transposes, O = P @ V, normalize, store."""
            PT = ptp.tile([P, NVT, P], f16, tag="PT", name=f"PT{b}_{qi}")
            for j in range(NVT):
                pst = psum_t.tile([P, P], f16, tag="pst", name=f"ppt{b}_{qi}_{j}")
                nc.tensor.transpose(pst, Pt[:, j * P : (j + 1) * P], ident)
                nc.vector.tensor_copy(PT[:, j, :], pst)
            psO = psum.tile([P, D], f32, tag="ps", name=f"psO{b}_{qi}")
            out_sb = outp.tile([P, D], f32, tag="out", name=f"o{b}_{qi}")
            for dch in range(NDCH):
                sl = slice(dch * DCH, (dch + 1) * DCH)
                for j in range(NVT):
                    nc.tensor.matmul(
                        psO[:, sl],
                        PT[:, j, :],
                        Vn[:, j, sl],
                        start=(j == 0),
                        stop=(j == NVT - 1),
                    )
                nc.vector.tensor_scalar_mul(out_sb[:, sl], psO[:, sl], rinv)
            nc.sync.dma_start(o_d[b, qi * P : (qi + 1) * P, :], out_sb)

        pending = None
        for b in range(BPC):
            VT, Vn = preload(b)
            for qi in range(NQT):
                if pending is not None:
                    pb, pq, ppsS, pstats, pVn = pending
                    Ppt, prinv = softmax_part(pb, pq, ppsS, pstats)
                psS, stats = head(b, qi, VT)
                if pending is not None:
                    tail(pb, pq, Ppt, prinv, pVn)
                pending = (b, qi, psS, stats, Vn)
        pb, pq, ppsS, pstats, pVn = pending
        Ppt, prinv = softmax_part(pb, pq, ppsS, pstats)
        tail(pb, pq, Ppt, prinv, pVn)

    nc.compile()
    return nc


_NC_CACHE = None


def _get_nc():
    global _NC_CACHE
    if _NC_CACHE is None:
        _NC_CACHE = build_nc()
    return _NC_CACHE


def kernel(query: np.ndarray, value: np.ndarray) -> np.ndarray:
    query = np.asarray(query)
    value = np.asarray(value)
    assert query.shape == (B, LQ, D) and value.shape == (B, LV, D)
    q16 = np.ascontiguousarray(query.astype(np.float16))
    v16 = np.ascontiguousarray(value.astype(np.float16))
    nc = _get_nc()
    in_maps = [
        {
            "q16": q16[i * BPC : (i + 1) * BPC],
            "v16": v16[i * BPC : (i + 1) * BPC],
        }
        for i in range(NCORES)
    ]
    res = run_bass_kernel_spmd(nc, in_maps, list(range(NCORES)))
    out = np.concatenate([res.results[i]["o"] for i in range(NCORES)], axis=0)
    return out


# revision 25
# speedup vs baseline: 1.2067x; 1.2067x over previous
"""TRN2 Bass kernel for batched dot-product attention (no scale, eval mode).

reference:
    score   = einsum('bqd,bvd->bqv', query, value)      # B=16, L=2048, D=1024
    attn    = softmax(score, axis=-1)
    context = einsum('bqv,bvd->bqd', attn, value)

Sharding: data-parallel over batch; each of 8 NeuronCores handles 2 batch
elements, no communication. Inputs are pre-cast to fp16 on the host (part of
the sharding/layout prep); matmuls run fp16 with fp32 PSUM accumulation.

Per-core per-batch plan:
  - preload V: natural fp16 copy Vn + transposed copy VT via PE transposes
  - per 128-row q-tile (software-pipelined, tail lags one tile):
      head: q tile load, QT via PE transposes, S = QT.T @ VT -> PSUM,
            per-chunk rowmax on DVE
      tail: exp(S - max) on ACT (fp16 P, fused rowsum), P^T via PE
            transposes, O = PT.T @ Vn -> PSUM, multiply by 1/rowsum, DMA out
"""

from contextlib import ExitStack

import numpy as np

import concourse.tile as tile
from concourse import bacc, mybir
from concourse.bass_utils import run_bass_kernel_spmd
from concourse.masks import make_identity

B, LQ, LV, D = 16, 2048, 2048, 1024
NCORES = 8
BPC = B // NCORES  # batches per core
P = 128
NQT = LQ // P  # 16 q tiles
NVT = LV // P  # 16 v tiles
ND = D // P  # 8 d tiles
VCH = 512  # MM1 moving-operand chunk (free dim)
NCH = LV // VCH  # 4
DCH = 512  # MM2 moving-operand chunk
NDCH = D // DCH  # 2

f32 = mybir.dt.float32
f16 = mybir.dt.float16
EXP = mybir.ActivationFunctionType.Exp
AX = mybir.AxisListType.X


def build_nc():
    nc = bacc.Bacc("TRN2", target_bir_lowering=False, debug=False)
    q_d = nc.dram_tensor("q16", [BPC, LQ, D], f16, kind="ExternalInput").ap()
    v_d = nc.dram_tensor("v16", [BPC, LV, D], f16, kind="ExternalInput").ap()
    o_d = nc.dram_tensor("o", [BPC, LQ, D], f32, kind="ExternalOutput").ap()

    with tile.TileContext(nc) as tc, ExitStack() as ctx:
        const = ctx.enter_context(tc.tile_pool(name="const", bufs=1))
        vpool = ctx.enter_context(tc.tile_pool(name="vpool", bufs=2))
        stage = ctx.enter_context(tc.tile_pool(name="stage", bufs=3))
        qtp = ctx.enter_context(tc.tile_pool(name="qtp", bufs=2))
        pp = ctx.enter_context(tc.tile_pool(name="pp", bufs=2))
        ptp = ctx.enter_context(tc.tile_pool(name="ptp", bufs=2))
        outp = ctx.enter_context(tc.tile_pool(name="outp", bufs=2))
        statp = ctx.enter_context(tc.tile_pool(name="statp", bufs=2))
        psum = ctx.enter_context(tc.tile_pool(name="psum", bufs=3, space="PSUM"))
        psum_t = ctx.enter_context(tc.tile_pool(name="psum_t", bufs=2, space="PSUM"))

        ident = const.tile([P, P], f16)
        make_identity(nc, ident)

        def preload(b):
            """V natural fp16 + VT via PE transposes."""
            VT = vpool.tile([P, ND, LV], f16, tag="VT", name=f"VT{b}")
            Vn = vpool.tile([P, NVT, D], f16, tag="Vn", name=f"Vn{b}")
            for j in range(NVT):
                nc.gpsimd.dma_start(
                    out=Vn[:, j, :], in_=v_d[b, j * P : (j + 1) * P, :]
                )
                pst8 = psum_t.tile([P, ND, P], f16, tag="pst8", name=f"pvt{b}_{j}")
                for k in range(ND):
                    nc.tensor.transpose(
                        pst8[:, k, :], Vn[:, j, k * P : (k + 1) * P], ident
                    )
                nc.scalar.copy(VT[:, :, j * P : (j + 1) * P], pst8)
            return VT, Vn

        def head(b, qi, VT):
            """Load+transpose Q tile, S = Q @ V^T, per-chunk row maxes."""
            q16 = stage.tile([P, D], f16, tag="q16", name=f"q16_{b}_{qi}")
            nc.gpsimd.dma_start(out=q16, in_=q_d[b, qi * P : (qi + 1) * P, :])
            QT = qtp.tile([P, ND, P], f16, tag="QT", name=f"QT{b}_{qi}")
            pst8 = psum_t.tile([P, ND, P], f16, tag="pst8", name=f"pqt{b}_{qi}")
            for k in range(ND):
                nc.tensor.transpose(pst8[:, k, :], q16[:, k * P : (k + 1) * P], ident)
            nc.scalar.copy(QT, pst8)

            psS = [
                psum.tile([P, 2 * VCH], f32, tag="ps", name=f"psS{b}_{qi}_{h}")
                for h in range(NCH // 2)
            ]
            stats = statp.tile([P, NCH], f32, tag="stats", name=f"st{b}_{qi}")
            for n in range(NCH):
                sl = slice((n % 2) * VCH, (n % 2 + 1) * VCH)
                for k in range(ND):
                    nc.tensor.matmul(
                        psS[n // 2][:, sl],
                        QT[:, k, :],
                        VT[:, k, n * VCH : (n + 1) * VCH],
                        start=(k == 0),
                        stop=(k == ND - 1),
                    )
                nc.vector.reduce_max(stats[:, n : n + 1], psS[n // 2][:, sl], axis=AX)
            return psS, stats

        def softmax_part(b, qi, psS, stats):
            """Softmax on ACT/DVE: exp(S - max) -> fp16 P, plus 1/rowsum."""
            negmax = statp.tile([P, 1], f32, tag="negmax", name=f"nm{b}_{qi}")
            nc.vector.reduce_max(negmax, stats, axis=AX, negate=True)
            sums = statp.tile([P, NCH], f32, tag="sums", name=f"sm{b}_{qi}")
            Pt = pp.tile([P, LV], f16, tag="P", name=f"P{b}_{qi}")
            for n in range(NCH):
                sl = slice((n % 2) * VCH, (n % 2 + 1) * VCH)
                nc.scalar.activation(
                    Pt[:, n * VCH : (n + 1) * VCH],
                    psS[n // 2][:, sl],
                    EXP,
                    bias=negmax,
                    accum_out=sums[:, n : n + 1],
                )
            rowsum = statp.tile([P, 1], f32, tag="rowsum", name=f"rs{b}_{qi}")
            nc.vector.reduce_sum(rowsum, sums, axis=AX)
            rinv = statp.tile([P, 1], f32, tag="rinv", name=f"ri{b}_{qi}")
            nc.vector.reciprocal(rinv, rowsum)
            return Pt, rinv

        def tail(b, qi, Pt, rinv, Vn):
            """P^T via PE transposes, O = P @ V, normalize, store."""
            PT = ptp.tile([P, NVT, P], f16, tag="PT", name=f"PT{b}_{qi}")
            for hf in range(2):
                pst8 = psum_t.tile(
                    [P, ND, P], f16, tag="pst8", name=f"ppt{b}_{qi}_{hf}"
                )
                for t in range(ND):
                    j = hf * ND + t
                    nc.tensor.transpose(
                        pst8[:, t, :], Pt[:, j * P : (j + 1) * P], ident
                    )
                nc.vector.tensor_copy(PT[:, hf * ND : (hf + 1) * ND, :], pst8)
            psO = psum.tile([P, D], f32, tag="ps", name=f"psO{b}_{qi}")
            out_sb = outp.tile([P, D], f32, tag="out", name=f"o{b}_{qi}")
            for dch in range(NDCH):
                sl = slice(dch * DCH, (dch + 1) * DCH)
                for j in range(NVT):
                    nc.tensor.matmul(
                        psO[:, sl],
                        PT[:, j, :],
                        Vn[:, j, sl],
                        start=(j == 0),
                        stop=(j == NVT - 1),
                    )
                nc.vector.tensor_scalar_mul(out_sb[:, sl], psO[:, sl], rinv)
            nc.sync.dma_start(o_d[b, qi * P : (qi + 1) * P, :], out_sb)

        pending = None
        for b in range(BPC):
            VT, Vn = preload(b)
            for qi in range(NQT):
                if pending is not None:
                    pb, pq, ppsS, pstats, pVn = pending
                    Ppt, prinv = softmax_part(pb, pq, ppsS, pstats)
                psS, stats = head(b, qi, VT)
                if pending is not None:
                    tail(pb, pq, Ppt, prinv, pVn)
                pending = (b, qi, psS, stats, Vn)
        pb, pq, ppsS, pstats, pVn = pending
        Ppt, prinv = softmax_part(pb, pq, ppsS, pstats)
        tail(pb, pq, Ppt, prinv, pVn)

    nc.compile()
    return nc


_NC_CACHE = None


def _get_nc():
    global _NC_CACHE
    if _NC_CACHE is None:
        _NC_CACHE = build_nc()
    return _NC_CACHE


def kernel(query: np.ndarray, value: np.ndarray) -> np.ndarray:
    query = np.asarray(query)
    value = np.asarray(value)
    assert query.shape == (B, LQ, D) and value.shape == (B, LV, D)
    q16 = np.ascontiguousarray(query.astype(np.float16))
    v16 = np.ascontiguousarray(value.astype(np.float16))
    nc = _get_nc()
    in_maps = [
        {
            "q16": q16[i * BPC : (i + 1) * BPC],
            "v16": v16[i * BPC : (i + 1) * BPC],
        }
        for i in range(NCORES)
    ]
    res = run_bass_kernel_spmd(nc, in_maps, list(range(NCORES)))
    out = np.concatenate([res.results[i]["o"] for i in range(NCORES)], axis=0)
    return out
